# revision 39
# baseline (speedup 1.0000x reference)
"""Trainium2 Bass kernel: scatter rows of packed upper-triangle data into
[B, 2048, 2048] matrices (zeros in the strict lower triangle).

Strategy (pure data parallel over batch, 4 rows per core on 8 cores):
  The packed triu row i lives at flat offset start_i = i*2048 - i*(i-1)/2
  with length 2048-i.  For a 128-row block (rows r0..r0+127, r0=128*bi),
  loading from offset (start_{r0+p} - p) with fixed width W = 2048-r0 puts
  element j of partition p at matrix column r0+j, correctly aligned for all
  columns >= row index; only the first p elements of each partition (the
  below-diagonal part of the leading 128x128 diagonal block) are junk.
  One indirect (gather) DMA per block-row therefore loads the whole
  right-of-diagonal rectangle; a 128x128 triu-mask multiply zeroes the junk;
  one rectangular store writes rows [r0:r0+128], cols [r0:2048].
  Strictly-lower 128x128 blocks are never written: the PJRT runner donates
  zero-initialized output buffers (a contract run_bass_via_pjrt documents);
  a host-side spot-check + explicit zero-fill fallback guards that.

Scheduling (variant v4, this session): gather (read) and store (write) DMA
streams are PHASE-SEPARATED instead of run concurrently.  Measured on this
part: read-only 70us/iter, write-only 85us/iter, but concurrent mix 218us >
70+85 — mixed read/write traffic pays a large HBM/SDMA direction-turnaround
tax.  v4 issues each batch-element's 16 gathers as a burst, then its 16
stores in REVERSE block order: with the tile pool exactly 16 buffers deep,
the next group's first gather carries a WAR dependency on the last store to
drain, and the first store (head-of-line on its HWDGE ring) waits on the
last gather — so read and write bursts alternate strictly.  Single-core this
hits the 155-165us serialization floor; 8-core it gave 198us vs 218us
baseline under like-for-like conditions (cross-core phase alignment is not
controllable cheaply — an NRT rendezvous start barrier adds unstable
multi-ms latency — so the remaining ~40us of chip-level read/write mixing
across unaligned cores is left on the table).

Failed directions (measured): bf16 cast-store via SWDGE (260us — Q7
descriptor-gen serializes all 128 SWDGE instructions; the indirect CAST
gather variant crashes HW outright, see _body_vb); 3-way store rings via
gpsimd (249us); 32-block phases (223us — longer unaligned phases mix worse
chip-wide); balanced-byte ring split (tied with alternation); per-group or
per-iteration all_core_barrier (deadlocks the Tile scheduler inside For_i).
"""

import numpy as np

MATSIZE = 2048
TRIU_LEN = MATSIZE * (MATSIZE + 1) // 2  # 2098176
N_CORES = 8
B_FULL = 32
B_LOCAL = B_FULL // N_CORES  # 4
P = 128
NB = MATSIZE // P  # 16 block-rows

_CACHE = {}

# Final configuration used by kernel() (and test.py's timing harness).
# v4 = phase-separated pipeline: 16-gather read burst, then the 16 stores in
# reverse order (WAR buffer reuse + head-of-line queue waits enforce strict
# read/write phase alternation; concurrent read+write streams measured up to
# 40% slower than serializing them on this part).
KERNEL_KWARGS = {"variant": "v4", "bufs": 16}
# Previous session's config (concurrent streams): {"bufs": 16, "alt_store": True}
TABLE_KWARGS = {}


def _make_tables(combine_batch=False, group=None):
    i = np.arange(MATSIZE, dtype=np.int64)
    starts = i * MATSIZE - (i * (i - 1)) // 2  # start offset of triu row i
    p = np.arange(P)
    # idx[p, bi] = starts[128*bi + p] - p
    idx = (starts.reshape(NB, P).T - p[:, None]).astype(np.int32)
    if combine_batch:
        group = B_LOCAL
    if group:
        # idx[p, bi*group + k] = k*TRIU_LEN + starts[128*bi + p] - p
        k = np.arange(group, dtype=np.int64) * TRIU_LEN
        idx = (
            idx[:, :, None].astype(np.int64) + k[None, None, :]
        ).reshape(P, NB * group).astype(np.int32)
    mask = np.triu(np.ones((P, P), dtype=np.float32))
    return idx, mask


def _build_nc(
    repeat=1, bufs=8, variant="v1", order="b_outer", num_swdge_queues=1, loop=1,
    alt_store=False, store_queue=None, bufs_u=None, group=NB, rot=0,
):
    import functools

    import concourse.bacc as bacc
    import concourse.mybir as mybir
    from concourse import bass
    from concourse.tile import TileContext

    f32 = mybir.dt.float32
    i32 = mybir.dt.int32

    idx_cols = {
        "v1": NB, "v2": NB * B_LOCAL, "v3": NB * 2,
        "vb": NB, "vbu": NB, "vb2": NB * B_LOCAL,
        "lo": NB, "so": NB, "c1": NB, "ls": NB, "v1s": NB, "v4": NB, "v5": NB,
        "v6": NB,
    }[variant]
    nc = bacc.Bacc(
        "TRN2",
        target_bir_lowering=False,
        debug=False,
        num_swdge_queues=num_swdge_queues,
        num_devices=(
            N_CORES
            if variant in ("v4", "v5") and alt_store in ("start", "iter", "group")
            else None
        ),
    )
    x = nc.dram_tensor("x", [1, B_LOCAL * TRIU_LEN], f32, kind="ExternalInput")
    idx = nc.dram_tensor("idx", [P, idx_cols], i32, kind="ExternalInput")
    mask = nc.dram_tensor("mask", [P, P], f32, kind="ExternalInput")
    y = nc.dram_tensor("y", [B_LOCAL * MATSIZE, MATSIZE], f32, kind="ExternalOutput")

    from contextlib import ExitStack

    if alt_store == "start":
        # One-time cross-core start alignment: a prelude AllGather (inserted
        # by compile() right after the gpsimd preamble, i.e. before this
        # wait) plus this wait keep all 8 cores' read/write phases launched
        # in lockstep.  Emitted BEFORE TileContext so both live in the entry
        # block, which the Tile scheduler's deadlock-check sim doesn't
        # simulate (a single simulated core can never complete an 8-core
        # collective).
        nc.gpsimd.bir_kernel_barrier_wait(replica_groups=[list(range(N_CORES))])

    with TileContext(nc) as tc:
        with ExitStack() as stack:
            cpool = stack.enter_context(tc.tile_pool(name="const", bufs=1))
            dpool = stack.enter_context(tc.tile_pool(name="data", bufs=bufs))
            idx_t = cpool.tile(list(idx.shape), i32)
            nc.sync.dma_start(out=idx_t[:], in_=idx[:])
            mask_t = cpool.tile([P, P], f32)
            nc.sync.dma_start(out=mask_t[:], in_=mask[:])
            if variant in ("so", "ls"):
                src_t = cpool.tile([P, MATSIZE], f32)
                nc.sync.dma_start(
                    out=src_t[:],
                    in_=x[:, : P * MATSIZE].rearrange("o (p c) -> p c", p=P),
                )
            if variant == "c1":
                upool = stack.enter_context(
                    tc.tile_pool(name="udata", bufs=bufs_u or bufs)
                )

            body = {
                "v1": functools.partial(_body_v1, alt_store=alt_store),
                "v2": _body_v2,
                "v3": _body_v3,
                "vb": _body_vb,
                "vbu": _body_vbu,
                "vb2": _body_vb2,
                "lo": _body_lo,
                "so": lambda *a: _body_so(*a, src_t=src_t, alt_store=alt_store),
                "ls": lambda *a: _body_ls(*a, src_t=src_t, alt_store=alt_store),
                "v1s": functools.partial(_body_v1s, alt_store=alt_store),
                "v4": functools.partial(
                    _body_v4, group=group, three_way=alt_store == "3way"
                ),
                "v5": functools.partial(
                    _body_v5, group=group, rot=rot, barrier=alt_store
                ),
                "v6": _body_v6,
                "c1": lambda *a: _body_c1(
                    *a, upool=upool, store_queue=store_queue
                ),
            }[variant]
            if loop > 1:
                with tc.For_i(0, loop, 1):
                    body(nc, bass, mybir, dpool, x, y, idx_t, mask_t, order)
            else:
                for _rep in range(repeat):
                    body(nc, bass, mybir, dpool, x, y, idx_t, mask_t, order)
    nc.compile()
    return nc


def _iter_order(order, n_inner):
    pairs = [(b, bi) for b in range(n_inner) for bi in range(NB)]
    if order == "bi_outer":
        pairs = [(b, bi) for bi in range(NB) for b in range(n_inner)]
    return pairs


def _body_v1(nc, bass, mybir, dpool, x, y, idx_t, mask_t, order="b_outer",
             alt_store=False):
    f32 = mybir.dt.float32
    for n, (b, bi) in enumerate(_iter_order(order, B_LOCAL)):
        r0 = bi * P
        W = MATSIZE - r0
        t = dpool.tile([P, W], f32, tag="t")
        nc.gpsimd.indirect_dma_start(
            out=t[:, :],
            out_offset=None,
            in_=x[:, :],
            in_offset=bass.IndirectOffsetOnAxis(ap=idx_t[:, bi : bi + 1], axis=1),
            element_offset=b * TRIU_LEN,
        )
        nc.vector.tensor_tensor(
            out=t[:, 0:P],
            in0=t[:, 0:P],
            in1=mask_t[:],
            op=mybir.AluOpType.mult,
        )
        # Alternate stores across the two physical HWDGE rings (SP / ACT).
        eng = nc.scalar if (alt_store and n % 2) else nc.sync
        eng.dma_start(
            out=y[b * MATSIZE + r0 : b * MATSIZE + r0 + P, r0:MATSIZE],
            in_=t[:, :],
        )


def _body_v2(nc, bass, mybir, dpool, x, y, idx_t, mask_t, order="b_outer"):
    """All B_LOCAL batch elements of one block-row in a single gather/store."""
    f32 = mybir.dt.float32
    y3 = y[:].rearrange("(k r) c -> r k c", k=B_LOCAL)
    for bi in range(NB):
        r0 = bi * P
        W = MATSIZE - r0
        t = dpool.tile([P, B_LOCAL * W], f32, tag="t")
        nc.gpsimd.indirect_dma_start(
            out=t[:, :],
            out_offset=None,
            in_=x[:, :],
            in_offset=bass.IndirectOffsetOnAxis(
                ap=idx_t[:, bi * B_LOCAL : (bi + 1) * B_LOCAL], axis=1
            ),
            element_offset=0,
        )
        tv = t[:, :].rearrange("p (k j) -> p k j", k=B_LOCAL)
        for k in range(B_LOCAL):
            nc.vector.tensor_tensor(
                out=tv[:, k, 0:P],
                in0=tv[:, k, 0:P],
                in1=mask_t[:],
                op=mybir.AluOpType.mult,
            )
        nc.sync.dma_start(
            out=y3[r0 : r0 + P, :, r0:MATSIZE],
            in_=tv[:, :, :],
        )


def _body_v3(nc, bass, mybir, dpool, x, y, idx_t, mask_t, order="b_outer"):
    """Pairs of batch elements per gather (256 descriptors); per-batch stores."""
    f32 = mybir.dt.float32
    for g, bi in _iter_order(order, B_LOCAL // 2):
        r0 = bi * P
        W = MATSIZE - r0
        t = dpool.tile([P, 2 * W], f32, tag="t")
        nc.gpsimd.indirect_dma_start(
            out=t[:, :],
            out_offset=None,
            in_=x[:, :],
            in_offset=bass.IndirectOffsetOnAxis(
                ap=idx_t[:, bi * 2 : bi * 2 + 2], axis=1
            ),
            element_offset=g * 2 * TRIU_LEN,
        )
        for k in range(2):
            b = g * 2 + k
            nc.vector.tensor_tensor(
                out=t[:, k * W : k * W + P],
                in0=t[:, k * W : k * W + P],
                in1=mask_t[:],
                op=mybir.AluOpType.mult,
            )
            nc.sync.dma_start(
                out=y[b * MATSIZE + r0 : b * MATSIZE + r0 + P, r0:MATSIZE],
                in_=t[:, k * W : (k + 1) * W],
            )


def _body_vb(nc, bass, mybir, dpool, x, y, idx_t, mask_t, order="b_outer"):
    """DO NOT USE ON HARDWARE: the indirect casting gather (f32 DRAM -> bf16
    SBUF) passes CoreSim but crashes real TRN2 with
    NRT_EXEC_UNIT_UNRECOVERABLE.  Kept only as a record of the experiment
    (2026-08-08); see vb2/vbu which share the same fatal gather."""
    bf16 = mybir.dt.bfloat16
    for n, (b, bi) in enumerate(_iter_order(order, B_LOCAL)):
        r0 = bi * P
        W = MATSIZE - r0
        t = dpool.tile([P, W], bf16, tag="t")
        nc.gpsimd.indirect_dma_start(
            out=t[:, :],
            out_offset=None,
            in_=x[:, :],
            in_offset=bass.IndirectOffsetOnAxis(ap=idx_t[:, bi : bi + 1], axis=1),
            element_offset=b * TRIU_LEN,
        )
        nc.vector.tensor_tensor(
            out=t[:, 0:P],
            in0=t[:, 0:P],
            in1=mask_t[:],
            op=mybir.AluOpType.mult,
        )
        nc.gpsimd.dma_start(
            out=y[b * MATSIZE + r0 : b * MATSIZE + r0 + P, r0:MATSIZE],
            in_=t[:, :],
        )


def _body_vb2(nc, bass, mybir, dpool, x, y, idx_t, mask_t, order="b_outer"):
    """vb with all B_LOCAL batch elements batched per SWDGE instruction: one
    512-descriptor cast-gather and one 512-descriptor cast-store per
    block-row (32 SWDGE instructions total instead of 128, amortizing the
    ~1us per-instruction desc-gen overhead)."""
    bf16 = mybir.dt.bfloat16
    y3 = y[:].rearrange("(k r) c -> r k c", k=B_LOCAL)
    for bi in range(NB):
        r0 = bi * P
        W = MATSIZE - r0
        t = dpool.tile([P, B_LOCAL * W], bf16, tag="t")
        nc.gpsimd.indirect_dma_start(
            out=t[:, :],
            out_offset=None,
            in_=x[:, :],
            in_offset=bass.IndirectOffsetOnAxis(
                ap=idx_t[:, bi * B_LOCAL : (bi + 1) * B_LOCAL], axis=1
            ),
            element_offset=0,
        )
        tv = t[:, :].rearrange("p (k j) -> p k j", k=B_LOCAL)
        for k in range(B_LOCAL):
            nc.vector.tensor_tensor(
                out=tv[:, k, 0:P],
                in0=tv[:, k, 0:P],
                in1=mask_t[:],
                op=mybir.AluOpType.mult,
            )
        nc.gpsimd.dma_start(
            out=y3[r0 : r0 + P, :, r0:MATSIZE],
            in_=tv[:, :, :],
        )


def _body_vbu(nc, bass, mybir, dpool, x, y, idx_t, mask_t, order="b_outer"):
    """bf16 gather + on-chip upcast: vector does mask-mult (bf16->f32) on the
    diagonal 128 cols and a copy upcast on the rest; stores stay on the two
    HWDGE rings in f32."""
    f32 = mybir.dt.float32
    bf16 = mybir.dt.bfloat16
    for n, (b, bi) in enumerate(_iter_order(order, B_LOCAL)):
        r0 = bi * P
        W = MATSIZE - r0
        t = dpool.tile([P, W], bf16, tag="t")
        u = dpool.tile([P, W], f32, tag="u")
        nc.gpsimd.indirect_dma_start(
            out=t[:, :],
            out_offset=None,
            in_=x[:, :],
            in_offset=bass.IndirectOffsetOnAxis(ap=idx_t[:, bi : bi + 1], axis=1),
            element_offset=b * TRIU_LEN,
        )
        nc.vector.tensor_tensor(
            out=u[:, 0:P],
            in0=t[:, 0:P],
            in1=mask_t[:],
            op=mybir.AluOpType.mult,
        )
        if W > P:
            nc.vector.tensor_scalar(
                out=u[:, P:W],
                in0=t[:, P:W],
                scalar1=1.0,
                scalar2=None,
                op0=mybir.AluOpType.mult,
            )
        eng = nc.scalar if n % 2 else nc.sync
        eng.dma_start(
            out=y[b * MATSIZE + r0 : b * MATSIZE + r0 + P, r0:MATSIZE],
            in_=u[:, :],
        )


def _body_lo(nc, bass, mybir, dpool, x, y, idx_t, mask_t, order="b_outer"):
    """Probe: gathers only (no mask, no store) — measures pure load-side DMA."""
    f32 = mybir.dt.float32
    for b, bi in _iter_order(order, B_LOCAL):
        r0 = bi * P
        W = MATSIZE - r0
        t = dpool.tile([P, W], f32, tag="t")
        nc.gpsimd.indirect_dma_start(
            out=t[:, :],
            out_offset=None,
            in_=x[:, :],
            in_offset=bass.IndirectOffsetOnAxis(ap=idx_t[:, bi : bi + 1], axis=1),
            element_offset=b * TRIU_LEN,
        )


def _body_so(nc, bass, mybir, dpool, x, y, idx_t, mask_t, order="b_outer",
             src_t=None, alt_store=False):
    """Probe: stores only (from a constant preloaded tile) — pure store-side DMA.
    Output is garbage; never use for correctness."""
    for n, (b, bi) in enumerate(_iter_order(order, B_LOCAL)):
        r0 = bi * P
        W = MATSIZE - r0
        eng = nc.scalar if (alt_store and n % 2) else nc.sync
        eng.dma_start(
            out=y[b * MATSIZE + r0 : b * MATSIZE + r0 + P, r0:MATSIZE],
            in_=src_t[:, :W],
        )


def _body_ls(nc, bass, mybir, dpool, x, y, idx_t, mask_t, order="b_outer",
             src_t=None, alt_store=False):
    """Probe: gathers AND stores with NO data dependency between them —
    isolates resource contention (SDMA/SBUF/HBM) from pipeline stalls.
    Output is garbage; never use for correctness."""
    f32 = mybir.dt.float32
    for n, (b, bi) in enumerate(_iter_order(order, B_LOCAL)):
        r0 = bi * P
        W = MATSIZE - r0
        t = dpool.tile([P, W], f32, tag="t")
        nc.gpsimd.indirect_dma_start(
            out=t[:, :],
            out_offset=None,
            in_=x[:, :],
            in_offset=bass.IndirectOffsetOnAxis(ap=idx_t[:, bi : bi + 1], axis=1),
            element_offset=b * TRIU_LEN,
        )
        eng = nc.scalar if (alt_store and n % 2) else nc.sync
        eng.dma_start(
            out=y[b * MATSIZE + r0 : b * MATSIZE + r0 + P, r0:MATSIZE],
            in_=src_t[:, :W],
        )


def _body_v1s(nc, bass, mybir, dpool, x, y, idx_t, mask_t, order="b_outer",
              alt_store=False):
    """v1 with the DVE mask op taken OFF the big store's critical path: the
    mask-mult writes the diagonal 128x128 block to a separate small tile, so
    the [128, W-128] bulk store depends only on its gather; only the small
    diagonal store waits for DVE."""
    f32 = mybir.dt.float32
    for n, (b, bi) in enumerate(_iter_order(order, B_LOCAL)):
        r0 = bi * P
        W = MATSIZE - r0
        row0 = b * MATSIZE + r0
        t = dpool.tile([P, W], f32, tag="t")
        d = dpool.tile([P, P], f32, tag="d")
        nc.gpsimd.indirect_dma_start(
            out=t[:, :],
            out_offset=None,
            in_=x[:, :],
            in_offset=bass.IndirectOffsetOnAxis(ap=idx_t[:, bi : bi + 1], axis=1),
            element_offset=b * TRIU_LEN,
        )
        nc.vector.tensor_tensor(
            out=d[:, :],
            in0=t[:, 0:P],
            in1=mask_t[:],
            op=mybir.AluOpType.mult,
        )
        eng = nc.scalar if (alt_store and n % 2) else nc.sync
        eng2 = nc.sync if (alt_store and n % 2) else nc.scalar
        if W > P:
            eng.dma_start(
                out=y[row0 : row0 + P, r0 + P : MATSIZE],
                in_=t[:, P:W],
            )
        eng2.dma_start(
            out=y[row0 : row0 + P, r0 : r0 + P],
            in_=d[:, :],
        )


def _body_v4(nc, bass, mybir, dpool, x, y, idx_t, mask_t, order="b_outer",
             group=NB, three_way=False):
    """Phase-separated pipeline: emit `group` gathers (each followed by its
    in-place DVE mask), then the `group` stores in REVERSE order.  Requires
    the t-pool to have exactly `group` buffers: the next phase's first gather
    (head-of-line on the SWDGE queue) then carries a WAR dependency on the
    LAST store to drain, and the first store (head-of-line on its HWDGE ring)
    waits for the last gather — so read-bursts and write-bursts alternate
    strictly instead of running concurrently.  Concurrent mixing measured 40%
    slower than serializing the two streams (218us vs lo+so=155us per iter).
    """
    f32 = mybir.dt.float32
    pairs = _iter_order(order, B_LOCAL)
    assert len(pairs) % group == 0
    for g, g0 in enumerate(range(0, len(pairs), group)):
        tiles = []
        for b, bi in pairs[g0 : g0 + group]:
            r0 = bi * P
            W = MATSIZE - r0
            t = dpool.tile([P, W], f32, tag="t")
            nc.gpsimd.indirect_dma_start(
                out=t[:, :],
                out_offset=None,
                in_=x[:, :],
                in_offset=bass.IndirectOffsetOnAxis(
                    ap=idx_t[:, bi : bi + 1], axis=1
                ),
                element_offset=b * TRIU_LEN,
            )
            nc.vector.tensor_tensor(
                out=t[:, 0:P],
                in0=t[:, 0:P],
                in1=mask_t[:],
                op=mybir.AluOpType.mult,
            )
            tiles.append((b, bi, t))
        for n, (b, bi, t) in enumerate(reversed(tiles)):
            r0 = bi * P
            W = MATSIZE - r0
            if three_way:
                eng = (nc.sync, nc.scalar, nc.gpsimd)[(n + g) % 3]
            else:
                eng = nc.scalar if (n + g) % 2 else nc.sync
            eng.dma_start(
                out=y[b * MATSIZE + r0 : b * MATSIZE + r0 + P, r0:MATSIZE],
                in_=t[:, :W],
            )


def _body_v5(nc, bass, mybir, dpool, x, y, idx_t, mask_t, order="b_outer",
             group=2 * NB, rot=0, barrier=False):
    """v4 with per-(bi, batch-parity) tile tags sized exactly [P, W(bi)], so
    TWO batches of tiles fit in SBUF (2 x 69.6KB/partition) -> 32-block
    phases, half the phase flips of v4.  `rot` rotates the reversed store
    order so the first `rot` stores depend on earlier gathers (filling the
    read->write flip bubble with useful writes) and the write->read flip can
    likewise start slightly early."""
    f32 = mybir.dt.float32
    pairs = _iter_order(order, B_LOCAL)
    assert len(pairs) % group == 0
    nslots = group // NB  # batches resident at once (1 or 2)
    if barrier == "iter":
        # Re-align all 8 cores once per iteration so their read/write phases
        # stay in lockstep chip-wide (drift remixes reads+writes at HBM).
        nc.all_core_barrier()
    for g, g0 in enumerate(range(0, len(pairs), group)):
        if barrier == "group":
            nc.all_core_barrier()
        tiles = []
        for b, bi in pairs[g0 : g0 + group]:
            r0 = bi * P
            W = MATSIZE - r0
            t = dpool.tile([P, W], f32, tag=f"t{bi}_{b % nslots}")
            nc.gpsimd.indirect_dma_start(
                out=t[:, :],
                out_offset=None,
                in_=x[:, :],
                in_offset=bass.IndirectOffsetOnAxis(
                    ap=idx_t[:, bi : bi + 1], axis=1
                ),
                element_offset=b * TRIU_LEN,
            )
            nc.vector.tensor_tensor(
                out=t[:, 0:P],
                in0=t[:, 0:P],
                in1=mask_t[:],
                op=mybir.AluOpType.mult,
            )
            tiles.append((b, bi, t))
        rev = list(reversed(tiles))
        order_st = rev[rot:] + rev[:rot] if rot else rev
        for n, (b, bi, t) in enumerate(order_st):
            r0 = bi * P
            W = MATSIZE - r0
            eng = nc.scalar if (n + g) % 2 else nc.sync
            eng.dma_start(
                out=y[b * MATSIZE + r0 : b * MATSIZE + r0 + P, r0:MATSIZE],
                in_=t[:, :W],
            )


# Perfectly byte-balanced split of block widths W(bi)=2048-128*bi across the
# two HWDGE rings: both subsets sum to 8704 columns (4.46 MB per phase each).
_RING_A_BIS = {0, 3, 4, 6, 8, 11, 13, 15}   # 2048+1664+1536+1280+1024+640+384+128
_RING_B_BIS = {1, 2, 5, 7, 9, 10, 12, 14}   # 1920+1792+1408+1152+896+768+512+256


def _body_v6(nc, bass, mybir, dpool, x, y, idx_t, mask_t, order="b_outer"):
    """v4 with the store-phase ring assignment chosen by byte-balance
    (exact 50/50 split across the two HWDGE rings) instead of alternation
    (which leaves a 0.53MB = ~2.5us tail imbalance per phase).  Store order
    within each ring stays reversed, preserving the strict read/write phase
    alternation via head-of-line waits and WAR buffer reuse."""
    f32 = mybir.dt.float32
    pairs = _iter_order(order, B_LOCAL)
    for g0 in range(0, len(pairs), NB):
        tiles = []
        for b, bi in pairs[g0 : g0 + NB]:
            r0 = bi * P
            W = MATSIZE - r0
            t = dpool.tile([P, W], f32, tag="t")
            nc.gpsimd.indirect_dma_start(
                out=t[:, :],
                out_offset=None,
                in_=x[:, :],
                in_offset=bass.IndirectOffsetOnAxis(
                    ap=idx_t[:, bi : bi + 1], axis=1
                ),
                element_offset=b * TRIU_LEN,
            )
            nc.vector.tensor_tensor(
                out=t[:, 0:P],
                in0=t[:, 0:P],
                in1=mask_t[:],
                op=mybir.AluOpType.mult,
            )
            tiles.append((b, bi, t))
        for b, bi, t in reversed(tiles):
            r0 = bi * P
            W = MATSIZE - r0
            eng = nc.sync if bi in _RING_A_BIS else nc.scalar
            eng.dma_start(
                out=y[b * MATSIZE + r0 : b * MATSIZE + r0 + P, r0:MATSIZE],
                in_=t[:, :W],
            )


def _body_c1(nc, bass, mybir, dpool, x, y, idx_t, mask_t, order="b_outer",
             upool=None, store_queue=None):
    """f32 gather -> DVE mask-mult + downcast to bf16 -> SWDGE cast-store
    (bf16 SBUF -> f32 HBM).  Halves the SBUF-side store bytes; tests whether
    the SBUF/SDMA fabric or the HBM side is the binding bandwidth."""
    f32 = mybir.dt.float32
    bf16 = mybir.dt.bfloat16
    for n, (b, bi) in enumerate(_iter_order(order, B_LOCAL)):
        r0 = bi * P
        W = MATSIZE - r0
        t = dpool.tile([P, W], f32, tag="t")
        u = upool.tile([P, W], bf16, tag="u")
        nc.gpsimd.indirect_dma_start(
            out=t[:, :],
            out_offset=None,
            in_=x[:, :],
            in_offset=bass.IndirectOffsetOnAxis(ap=idx_t[:, bi : bi + 1], axis=1),
            element_offset=b * TRIU_LEN,
        )
        nc.vector.tensor_tensor(
            out=u[:, 0:P],
            in0=t[:, 0:P],
            in1=mask_t[:],
            op=mybir.AluOpType.mult,
        )
        if W > P:
            nc.vector.tensor_scalar(
                out=u[:, P:W],
                in0=t[:, P:W],
                scalar1=1.0,
                scalar2=None,
                op0=mybir.AluOpType.mult,
            )
        inst = nc.gpsimd.dma_start(
            out=y[b * MATSIZE + r0 : b * MATSIZE + r0 + P, r0:MATSIZE],
            in_=u[:, :],
        )
        if store_queue:
            inst.queue = f"qPoolDynamic{store_queue}"


def _get_nc():
    if "nc" not in _CACHE:
        _CACHE["nc"] = _build_nc(**KERNEL_KWARGS)
    return _CACHE["nc"]


def _zero_check_and_fix(out):
    """Unwritten strictly-lower 128x128 blocks rely on zero-donated output
    buffers; sample one element per such block per batch and zero-fill on
    host if the contract ever fails."""
    bis, bjs = np.tril_indices(NB, k=-1)
    samples = out[:, bis * P + 17, bjs * P + 3]
    if np.any(samples != 0.0):
        for bi in range(1, NB):
            out[:, bi * P : (bi + 1) * P, : bi * P] = 0.0
    return out


def kernel(**inputs) -> np.ndarray:
    from concourse.bass_utils import run_bass_kernel_spmd

    x_full = np.ascontiguousarray(np.asarray(inputs["inputs"], dtype=np.float32))
    assert x_full.shape == (B_FULL, TRIU_LEN)

    idx, mask = _make_tables(**TABLE_KWARGS)
    nc = _get_nc()

    in_maps = []
    for c in range(N_CORES):
        shard = x_full[c * B_LOCAL : (c + 1) * B_LOCAL].reshape(1, -1)
        in_maps.append({"x": shard, "idx": idx, "mask": mask})

    res = run_bass_kernel_spmd(nc, in_maps, list(range(N_CORES)))
    out = np.concatenate(
        [r["y"].reshape(B_LOCAL, MATSIZE, MATSIZE) for r in res.results], axis=0
    )
    return _zero_check_and_fix(out)

